# revision 1
# baseline (speedup 1.0000x reference)
"""ChannelGate (topk_masking) Trainium2 Bass kernel — v2.

Data parallel over batch (B=32 -> 4 samples x 8 cores), bf16 I/O.
Per core, per sample (x as 4 c-tiles [128, 3136] bf16):
  stats: DVE ts+accum (channel sum), in-place TT max tree (channel max),
         TT combines + GPSIMD partition_all_reduce (pixel max),
         PE ones-matmul (pixel sum, psum [128,1568] rounds).
  topk:  batched [8,512] f32 max8/match_replace sort (32 iters), tiny MLP
         on PE (interleave folded into host-split even/odd W1).
  gate:  PE K=100 matmul fuses conv(im2col) + sigmoid-term + k2 bias and
         broadcasts spw to 128 partitions in PSUM; ACT sigmoid reads PSUM
         directly with per-partition scale sqw; DVE ts(+1) + TT mult; y bf16.
"""
import numpy as np
from contextlib import ExitStack

import concourse.bass as bass
import concourse.tile as tile
from concourse import bacc, mybir, bass_isa
from concourse import bass_utils

F32 = mybir.dt.float32
BF16 = mybir.dt.bfloat16
AF = mybir.ActivationFunctionType
ALU = mybir.AluOpType
AX = mybir.AxisListType

B, C, H, W = 32, 512, 56, 56
HW = H * W            # 3136
S = 4                 # samples per core
NCORES = 8
G = 4                 # c-tiles of 128 per sample
RED = 32              # MLP hidden
PW = 62               # padded conv map width/height
NEG = -1.0e30
HHW = 1568            # half of HW
# 512-bank-aligned slices within a 1568-wide psum round
R1 = [(0, 512), (512, 512), (1024, 512), (1536, 32)]
ROUNDS = [0, HHW]


def build_program():
    nc = bacc.Bacc("TRN2", target_bir_lowering=False, debug=False,
                   num_devices=NCORES)

    x_d = nc.dram_tensor("x", [S, C, HW], BF16, kind="ExternalInput")
    y_d = nc.dram_tensor("y", [S, C, HW], BF16, kind="ExternalOutput")
    w1e_d = nc.dram_tensor("w1e", [64, 4 * RED], F32, kind="ExternalInput")
    w1o_d = nc.dram_tensor("w1o", [64, 4 * RED], F32, kind="ExternalInput")
    w2t_d = nc.dram_tensor("w2t", [RED, C], F32, kind="ExternalInput")
    b1_d = nc.dram_tensor("b1c", [RED, 1], F32, kind="ExternalInput")
    b2_d = nc.dram_tensor("b2c", [128, G], F32, kind="ExternalInput")
    w98_d = nc.dram_tensor("w98", [98, 128], BF16, kind="ExternalInput")
    w2x_d = nc.dram_tensor("w2x", [128, 128], BF16, kind="ExternalInput")
    id_d = nc.dram_tensor("ident", [128, 128], F32, kind="ExternalInput")
    ssc_d = nc.dram_tensor("sortscale", [8, 1], F32, kind="ExternalInput")
    ones_d = nc.dram_tensor("ones128", [128, 128], BF16, kind="ExternalInput")
    pad_d = nc.dram_tensor("pad0", [S * 2 * PW * PW], BF16, kind="ExternalInput")
    onesrow_d = nc.dram_tensor("onesrow", [1, HW], BF16, kind="ExternalInput")

    with tile.TileContext(nc) as tc:
        with ExitStack() as ctx:
            build_core(ctx, tc, x_d, y_d, w1e_d, w1o_d, w2t_d, b1_d, b2_d,
                       w98_d, w2x_d, id_d, ssc_d, ones_d, pad_d, onesrow_d)
    nc.compile()
    return nc


def build_core(ctx, tc, x_d, y_d, w1e_d, w1o_d, w2t_d, b1_d, b2_d,
               w98_d, w2x_d, id_d, ssc_d, ones_d, pad_d, onesrow_d):
    nc = tc.nc

    cpool = ctx.enter_context(tc.tile_pool(name="consts", bufs=1))
    xt_pool = ctx.enter_context(tc.tile_pool(name="xt", bufs=5))
    xt2_pool = ctx.enter_context(tc.tile_pool(name="xt2", bufs=2))
    scr_pool = ctx.enter_context(tc.tile_pool(name="scr", bufs=2))
    mA_pool = ctx.enter_context(tc.tile_pool(name="mA", bufs=1))
    mB_pool = ctx.enter_context(tc.tile_pool(name="mB", bufs=1))
    mC_pool = ctx.enter_context(tc.tile_pool(name="mC", bufs=2))
    px_pool = ctx.enter_context(tc.tile_pool(name="px", bufs=1))
    row_pool = ctx.enter_context(tc.tile_pool(name="rows", bufs=1))
    imt_pool = ctx.enter_context(tc.tile_pool(name="imt", bufs=4))
    sig_pool = ctx.enter_context(tc.tile_pool(name="sig", bufs=4))
    y_pool = ctx.enter_context(tc.tile_pool(name="yp", bufs=4))

    ps_gate = ctx.enter_context(tc.tile_pool(name="ps_gate", bufs=1,
                                             space="PSUM"))
    ps_small = ctx.enter_context(tc.tile_pool(name="ps_small", bufs=2,
                                              space="PSUM"))

    # ---- constants / weights in SBUF ----
    ident = cpool.tile([128, 128], F32)
    nc.sync.dma_start(ident[:], id_d.ap())
    ones128 = cpool.tile([128, 128], BF16)
    nc.sync.dma_start(ones128[:], ones_d.ap())
    w98 = cpool.tile([98, 128], BF16)
    nc.sync.dma_start(w98[:], w98_d.ap())
    w2x = cpool.tile([128, 128], BF16)
    nc.sync.dma_start(w2x[:], w2x_d.ap())
    w1e = cpool.tile([64, 4 * RED], F32)
    nc.sync.dma_start(w1e[:], w1e_d.ap())
    w1o = cpool.tile([64, 4 * RED], F32)
    nc.sync.dma_start(w1o[:], w1o_d.ap())
    w2t = cpool.tile([RED, C], F32)
    nc.sync.dma_start(w2t[:], w2t_d.ap())
    b1 = cpool.tile([RED, 1], F32)
    nc.sync.dma_start(b1[:], b1_d.ap())
    b2 = cpool.tile([128, G], F32)
    nc.sync.dma_start(b2[:], b2_d.ap())
    sortscale = cpool.tile([8, 1], F32)
    nc.sync.dma_start(sortscale[:], ssc_d.ap())

    sc = [cpool.tile([128, 8], F32, tag=f"sc{g}", name=f"scq{g}")
          for g in range(G)]
    srt = cpool.tile([8, C], F32)
    srtd = cpool.tile([8, 256], F32)
    tq = [cpool.tile([64, 8], F32, tag=f"tq{q}", name=f"tq{q}")
          for q in range(4)]
    h_sb = cpool.tile([RED, S], F32)
    sqw = [cpool.tile([128, S], F32, tag=f"sqw{g}", name=f"sqw{g}")
           for g in range(G)]
    ssrow = cpool.tile([128, HW], BF16)     # sample s at partition 32*s
    pxrow = cpool.tile([128, HW], BF16)
    xsig = cpool.tile([128, HW], BF16)      # sigmoid row 32s, ones row 32s+1
    prodall = cpool.tile([128, HW], BF16)
    nc.vector.memset(ssrow[:], 0.0)
    nc.vector.memset(pxrow[:], 0.0)

    # ================= PHASE 1: stats =================
    for s in range(S):
        xt = []
        for g in range(G):
            t = xt_pool.tile([128, HW], BF16, tag="t")
            nc.sync.dma_start(t[:], x_d.ap()[s, g * 128:(g + 1) * 128, :])
            xt.append(t)
            # channel sum + channel max via tensor_scalar accumulate (4x)
            scr = scr_pool.tile([128, HW], BF16)
            nc.vector.tensor_scalar(out=scr[:], in0=t[:], scalar1=1.0,
                                    scalar2=None, op0=ALU.mult, op1=ALU.add,
                                    accum_out=sc[g][:, s:s + 1])
            scr2 = scr_pool.tile([128, HW], BF16, tag="scr2", name="scr2")
            nc.vector.tensor_scalar(out=scr2[:], in0=t[:], scalar1=1.0,
                                    scalar2=None, op0=ALU.mult, op1=ALU.max,
                                    accum_out=sc[g][:, 4 + s:5 + s])

        # pixel max: combine 4 c-tiles, then partition all-reduce on gpsimd
        m01 = mA_pool.tile([128, HW], BF16)
        nc.vector.tensor_tensor(m01[:], xt[0][:], xt[1][:], op=ALU.max)
        m23 = mB_pool.tile([128, HW], BF16)
        nc.vector.tensor_tensor(m23[:], xt[2][:], xt[3][:], op=ALU.max)
        mall = mC_pool.tile([128, HW], BF16)
        nc.vector.tensor_tensor(mall[:], m01[:], m23[:], op=ALU.max)
        pxr = px_pool.tile([128, HW], BF16)
        nc.gpsimd.partition_all_reduce(pxr[:], mall[:], 128,
                                       bass_isa.ReduceOp.max)
        nc.vector.tensor_copy(pxrow[32 * s:32 * s + 1, :], pxr[0:1, :])

        # pixel sums: ones.T @ x (M=128 replicated), 2 psum rounds of 1568
        for roff in ROUNDS:
            pg = ps_gate.tile([128, HHW], F32, tag="gate")
            for (off, wdt) in R1:
                for g in range(G):
                    nc.tensor.matmul(pg[:, off:off + wdt], ones128[:],
                                     xt[g][:, roff + off:roff + off + wdt],
                                     start=(g == 0), stop=(g == G - 1))
            nc.scalar.copy(ssrow[32 * s:32 * s + 1, roff:roff + HHW],
                           pg[0:1, :])

    # ================= PHASE 2: topk sort + MLP =================
    for g in range(G):
        pst = ps_small.tile([8, 128], F32, tag='pss')
        nc.tensor.transpose(pst[:], sc[g][:], ident[:])
        nc.scalar.activation(srt[:, g * 128:(g + 1) * 128], pst[:], AF.Copy,
                             scale=sortscale[:])
    for it in range(32):
        m8 = srtd[:, 8 * it:8 * it + 8]
        nc.vector.max(out=m8, in_=srt[:])
        nc.vector.match_replace(out=srt[:], in_to_replace=m8,
                                in_values=srt[:], imm_value=NEG)
    for q in range(4):
        pst = ps_small.tile([64, 8], F32, tag='pss')
        nc.tensor.transpose(pst[:], srtd[:, 64 * q:64 * q + 64],
                            ident[0:8, 0:8])
        nc.scalar.copy(tq[q][:], pst[:])
    psh = ps_small.tile([RED, S], F32, tag='pss')
    for q in range(4):
        c0 = q * RED
        nc.tensor.matmul(psh[:], w1e[:, c0:c0 + RED], tq[q][:, 0:4],
                         start=(q == 0), stop=False)
        nc.tensor.matmul(psh[:], w1o[:, c0:c0 + RED], tq[q][:, 4:8],
                         start=False, stop=(q == 3))
    nc.scalar.activation(h_sb[:], psh[:], AF.Relu, bias=b1[:])
    for g in range(G):
        psm = ps_small.tile([128, S], F32, tag='pss')
        nc.tensor.matmul(psm[:], w2t[:, g * 128:(g + 1) * 128], h_sb[:],
                         start=True, stop=True)
        prod = cpool.tile([128, S], F32, tag=f"prod{g}")
        nc.vector.tensor_tensor(prod[:], sc[g][:, 0:4], sc[g][:, 4:8],
                                op=ALU.mult)
        sigp = cpool.tile([128, S], F32, tag=f"sigp{g}")
        nc.scalar.activation(sigp[:], prod[:], AF.Sigmoid, scale=1.0 / HW)
        nc.vector.tensor_tensor(sigp[:], sigp[:], psm[:], op=ALU.add)
        nc.scalar.activation(sqw[g][:], sigp[:], AF.Relu, bias=b2[:, g:g + 1])

    # ================= PHASE 3: spatial conv prep =================
    # batched sigmoid(ss1*ss2) for all 4 samples (rows 32s carry data)
    nc.vector.tensor_tensor(prodall[:], ssrow[:], pxrow[:], op=ALU.mult)
    nc.scalar.activation(xsig[:], prodall[:], AF.Sigmoid, scale=1.0 / C)
    imts = []
    for s in range(S):
        # ones row (k2 bias) overwrites sigmoid garbage at row 32s+1
        nc.sync.dma_start(xsig[32 * s + 1:32 * s + 2, :], onesrow_d.ap())
        # write padded interiors: ssrow (sums; /C folded in conv wts), pxrow
        for ci, src2 in ((0, ssrow), (1, pxrow)):
            base = ((s * 2 + ci) * PW + 3) * PW + 3
            dst = bass.AP(pad_d, base, [[PW, H], [1, W]])
            nc.gpsimd.dma_start(dst, src2[32 * s:32 * s + 1, :].rearrange(
                "p (h w) -> p h w", h=H))
        imt = imt_pool.tile([98, HW], BF16)
        for ci in range(2):
            for kh in range(7):
                base = ((s * 2 + ci) * PW + kh) * PW
                src = bass.AP(pad_d, base, [[1, 7], [PW, H], [1, W]])
                p0 = ci * 49 + kh * 7
                nc.sync.dma_start(imt[p0:p0 + 7, :], src)
        imts.append(imt)

    # ================= PHASE 4: gate =================
    for s in range(S):
        xg = []
        for g in range(G):
            t2 = xt2_pool.tile([128, HW], BF16, tag="t2")
            nc.sync.dma_start(t2[:], x_d.ap()[s, g * 128:(g + 1) * 128, :])
            xg.append(t2)
        sigs = [sig_pool.tile([128, HW], BF16, tag="sg", name=f"sg{s}_{g}")
                for g in range(G)]
        ys = [y_pool.tile([128, HW], BF16, tag="yg", name=f"yg{s}_{g}")
              for g in range(G)]
        for roff in ROUNDS:
            pg = ps_gate.tile([128, HHW], F32, tag="gate")
            imt_s = imts[s]
            for (off, wdt) in R1:
                nc.tensor.matmul(pg[:, off:off + wdt], w98[:],
                                 imt_s[:, roff + off:roff + off + wdt],
                                 start=True, stop=False)
                nc.tensor.matmul(
                    pg[:, off:off + wdt], w2x[32 * s:32 * s + 2, :],
                    xsig[32 * s:32 * s + 2, roff + off:roff + off + wdt],
                    start=False, stop=True, tile_position=(32 * s, 0))
            for g in range(G):
                sl = slice(roff, roff + HHW)
                nc.scalar.activation(sigs[g][:, sl], pg[:], AF.Sigmoid,
                                     scale=sqw[g][:, s:s + 1])
                nc.vector.tensor_scalar(out=sigs[g][:, sl],
                                        in0=sigs[g][:, sl], scalar1=1.0,
                                        scalar2=None, op0=ALU.add)
                nc.vector.tensor_tensor(ys[g][:, sl], sigs[g][:, sl],
                                        xg[g][:, sl], op=ALU.mult)
        for g in range(G):
            nc.sync.dma_start(y_d.ap()[s, g * 128:(g + 1) * 128, :], ys[g][:])


_NC_CACHE = {}


def _get_program():
    if "nc" not in _NC_CACHE:
        _NC_CACHE["nc"] = build_program()
    return _NC_CACHE["nc"]


def _host_params(w1, b1, w2, b2, conv_w, bn_gamma, bn_beta, bn_mean, bn_var):
    import ml_dtypes
    w1 = np.asarray(w1, np.float32)
    w2 = np.asarray(w2, np.float32)
    b1 = np.asarray(b1, np.float32)
    b2 = np.asarray(b2, np.float32)
    conv_w = np.asarray(conv_w, np.float32)

    w1e = np.ascontiguousarray(
        w1[:, 0::2].T.reshape(4, 64, RED).transpose(1, 0, 2).reshape(64, 4 * RED))
    w1o = np.ascontiguousarray(
        w1[:, 1::2].T.reshape(4, 64, RED).transpose(1, 0, 2).reshape(64, 4 * RED))
    w2t = np.ascontiguousarray(w2.T)                    # [32, 512]
    b1c = b1.reshape(RED, 1).copy()
    b2c = np.ascontiguousarray(b2.reshape(G, 128).T)    # [128, G]

    bn_scale = float(bn_gamma[0]) / np.sqrt(float(bn_var[0]) + 1e-5)
    k2 = float(bn_beta[0]) - float(bn_mean[0]) * bn_scale
    wcf = conv_w[0].astype(np.float64) * bn_scale       # [2, 7, 7]
    wcf = wcf.copy()
    wcf[0] /= C                                         # mean channel fold
    w98 = np.broadcast_to(wcf.reshape(98, 1), (98, 128)).astype(
        ml_dtypes.bfloat16).copy()
    w2x = np.zeros((128, 128), np.float32)
    for s in range(S):
        w2x[32 * s, :] = 1.0                            # sigmoid-term row
        w2x[32 * s + 1, :] = k2                         # BN bias row
    w2x = w2x.astype(ml_dtypes.bfloat16)

    sortscale = np.concatenate([np.full(4, 1.0 / HW, np.float32),
                                np.ones(4, np.float32)]).reshape(8, 1)
    ident = np.eye(128, dtype=np.float32)
    ones128 = np.ones((128, 128), ml_dtypes.bfloat16)
    pad0 = np.zeros(S * 2 * PW * PW, ml_dtypes.bfloat16)
    onesrow = np.ones((1, HW), ml_dtypes.bfloat16)
    return dict(w1e=w1e, w1o=w1o, w2t=w2t, b1c=b1c, b2c=b2c, w98=w98,
                w2x=w2x, ident=ident, sortscale=sortscale, pad0=pad0,
                ones128=ones128, onesrow=onesrow)


def kernel(x, w1, b1, w2, b2, conv_w, bn_gamma, bn_beta, bn_mean, bn_var):
    import ml_dtypes
    x = np.asarray(x, np.float32)
    params = _host_params(w1, b1, w2, b2, conv_w,
                          bn_gamma, bn_beta, bn_mean, bn_var)
    nc = _get_program()

    xr = x.reshape(B, C, HW).astype(ml_dtypes.bfloat16)
    in_maps = []
    for k in range(NCORES):
        m = {"x": np.ascontiguousarray(xr[k * S:(k + 1) * S])}
        m.update(params)
        in_maps.append(m)

    res = bass_utils.run_bass_kernel_spmd(nc, in_maps,
                                          core_ids=list(range(NCORES)))
    out = np.concatenate([res.results[k]["y"].astype(np.float32)
                          for k in range(NCORES)], axis=0)
    return out.reshape(B, C, H, W)

